# revision 34
# baseline (speedup 1.0000x reference)
"""LocallyConnected2d (non-overlapping 3x3 patches) Trainium2 kernel.

Problem: x [B=32, Cin=128, H=96, W=96], weight [Hout=32, Wout=32, Cout=128,
Cin=128, 3, 3], bias [Hout, Wout, Cout] -> out [B, Cout, Hout, Wout].

For each of the 1024 output positions (i, j) this is an independent
[B=32, K=1152] x [K=1152, Cout=128] matmul (K = Cin*KH*KW) plus bias.

Strategy:
  - Shard the 1024 positions over 8 NeuronCores by Hout rows (4 rows =
    128 positions per core).  The weight tensor (604 MB fp32) dominates,
    and position-sharding splits it evenly with zero duplication.
  - Host-side: cast x and weight to bf16 (halves the DMA bytes, which are
    the roofline) and rearrange so every DMA descriptor is a long
    contiguous run:  per-core layouts
        wk [kp=128, pos=128, ck=9, o=128]   (bf16)
        xk [kp=128, pos=128, ck=9, b=32]    (bf16)
    where the contraction index k = c*9 + p*3 + q is split as
    k = ck*128 + kp and kp sits on SBUF partitions.
  - Per position: 9 bf16 matmuls (lhsT = w chunk [128k x 128o] STATIONARY,
    which triggers the compiler-automatic Fast Weight Load since
    NumWeights==128 and dtype!=fp32; rhs = x chunk [128k x 32b] moving)
    accumulate into PSUM [128o, 32b]; a 10th bf16 matmul
    (bias[1,128] stationary x ones[1,32] moving) adds the bias.
    Keeping the whole PE stream bf16 avoids the 4x-slow fp32 path and
    the FP32HI FWL-disable erratum (measured: 210us -> ~25us PE time).
  - 16 positions share one PSUM bank [128, 512]; one DVE copy per bank
    moves results to an SBUF staging tile; 32-position staging tiles are
    DMA'd to DRAM densely (output layout [o, pos, b], transposed to
    [b, o, i, j] on host).
  - Input DMAs ride nc.sync (HWDGE ring 0), output DMAs ride nc.scalar
    (HWDGE ring 1) so a blocked store never head-of-line blocks a
    prefetch.
"""

import numpy as np
import ml_dtypes

import concourse.bass as bass
import concourse.bacc as bacc
import concourse.mybir as mybir
import concourse.tile as tile
from concourse.bass_utils import run_bass_kernel_spmd

KH = KW = 3
B, CIN, H, W_IN = 32, 128, 96, 96
HOUT, WOUT, COUT = 32, 32, 128
NCORES = 8
IPC = HOUT // NCORES          # Hout rows per core = 4
POS = IPC * WOUT              # positions per core = 128
K = CIN * KH * KW             # 1152
CK = K // 128                 # 9 k-chunks of 128

WG = 8     # positions per weight-DMA tile
XG = 16    # positions per x-DMA tile
PG = 16    # positions per PSUM bank
SG = 32    # positions per output staging tile
WBUFS = 8  # weight pool buffers (deep prefetch decouples DMA from PE stalls)
XBUFS = 4  # x pool buffers
PBUFS = 6  # PSUM pool buffers
X_ON_ACT = False  # legacy: issue x DMAs on the scalar (ACT) HWDGE ring
# Ring layout: per-core input DMA bandwidth is capped at ~356 GB/s TOTAL
# (shared across queues — measured: splitting rings does not scale), so the
# split below is about ordering, not bandwidth: w tiles never queue behind
# x tiles or behind output stores that wait on compute.
W_ENGS = ("sync",)   # weight DMAs: sync HWDGE ring only
X_ENG = "scalar"     # x DMAs: scalar HWDGE ring
O_ENG = "gpsimd"     # output stores + bias load: software DGE, off both rings
# Unified round-robin: every input DMA tile (w and x alike) takes the next
# ring in RR_RINGS, balancing bytes across rings.  Overrides W_ENGS/X_ENG
# when non-empty.  Each HWDGE ring sustains ~356 GB/s; the weight stream
# alone needs ~3 rings to keep up with the PE.
RR_RINGS = ()

BF16 = mybir.dt.bfloat16
FP32 = mybir.dt.float32
FP8E3 = mybir.dt.float8e3  # e3m4, max 15.5

# Weight dtype: fp8e3 halves the dominant weight DMA stream.  A single
# global scale s = W_TARGET/absmax(w) is applied to w before the fp8 cast
# and its inverse is folded into the bf16 x on the host, so the PE
# accumulates the true x.w in PSUM and nothing on-chip changes.
# Measured accuracy (exact host sim): rel_max 1.16e-2 vs 2e-2 gate.
W_FP8 = True
W_TARGET = 14.0
OUT_BF16 = False  # stage + store the output in bf16 (halves store bytes)
SEP_LDW = False   # separate InstLdweights + non-self-loading InstMatmult
# x in fp8e3 as well: per-position scale sx[pos]=14/absmax(x_patch) folded
# into w's quantization scale (w8 = e3m4(w*sw/sx[pos])), so PSUM = sw*(x.w)
# and a single global descale 1/sw (shipped as a [128,1] input, applied by
# the DVE tensor_scalar_mul on the PSUM->SBUF copy) recovers the output.
# Bias rides the PE at scale sw (bias_q = bias*sw).  Exact host sim:
# rel_max 1.65e-2 vs the 2e-2 gate.
X_FP8 = True

_NC_CACHE = {}


def set_config(**kw):
    g = globals()
    for k, v in kw.items():
        assert k in g, k
        g[k] = v
    _NC_CACHE.clear()


def _config_key():
    return (WG, XG, PG, SG, WBUFS, XBUFS, PBUFS, X_ON_ACT, W_FP8, W_ENGS,
            X_ENG, O_ENG, RR_RINGS, OUT_BF16, SEP_LDW, X_FP8)


def _build_bass(repeat=1, variant="full"):
    """Build the Bass program. repeat>1 wraps the body in a dynamic loop
    (identical work each trip) so wall-clock timing can amortize the axon
    dispatch overhead: T(repeat) ~= overhead + repeat * T_kernel.
    variant: "full" | "dma" (input DMAs only) | "pe" (no input DMAs) |
    "empty" (loop overhead calibration)."""
    key = ("nc", repeat, variant, _config_key())
    if key in _NC_CACHE:
        return _NC_CACHE[key]
    nc = bacc.Bacc()
    wdt = FP8E3 if W_FP8 else BF16
    xdt = FP8E3 if X_FP8 else BF16
    xk = nc.declare_dram_parameter("xk", [128, POS * CK * B], xdt, isOutput=False)
    wk = nc.declare_dram_parameter("wk", [128, POS * CK * COUT], wdt, isOutput=False)
    bk = nc.declare_dram_parameter("bk", [1, POS * COUT], BF16, isOutput=False)
    dsc = (
        nc.declare_dram_parameter("dsc", [128, 1], FP32, isOutput=False)
        if X_FP8
        else None
    )
    out = nc.declare_dram_parameter(
        "out", [COUT, POS * B], BF16 if OUT_BF16 else FP32, isOutput=True
    )

    XW = CK * B      # x columns per position = 288
    WW = CK * COUT   # w columns per position = 1152

    with tile.TileContext(nc) as tc:
        with (
            tc.tile_pool(name="wpool", bufs=WBUFS) as wpool,
            tc.tile_pool(name="xpool", bufs=XBUFS) as xpool,
            tc.tile_pool(name="spool", bufs=2) as spool,
            tc.tile_pool(name="cpool", bufs=1) as cpool,
            tc.tile_pool(name="ppool", bufs=PBUFS, space="PSUM") as ppool,
        ):
            ones = cpool.tile([1, B], BF16)
            nc.vector.memset(ones[:], 1.0)
            bias_t = cpool.tile([1, POS * COUT], BF16)
            # [1, N] DMAs run at 1/128th of ring bandwidth (single partition
            # line): keep this off the w/x rings.
            nc.gpsimd.dma_start(out=bias_t[:], in_=bk[:])
            dsc_t = None
            if dsc is not None:
                dsc_t = cpool.tile([128, 1], FP32)
                nc.gpsimd.dma_start(out=dsc_t[:], in_=dsc[:])

            def body():
                _emit_body(nc, tc, xk, wk, out, wpool, xpool, spool, ppool,
                           ones, bias_t, dsc_t, variant)

            if repeat == 1:
                body()
            else:
                with tc.For_i(0, repeat, 1):
                    body()
    nc.finalize()
    _NC_CACHE[key] = nc
    return nc


def _emit_body(nc, tc, xk, wk, out, wpool, xpool, spool, ppool, ones, bias_t,
               dsc_t, variant="full"):
    XW = CK * B
    WW = CK * COUT
    use_dma = variant in ("full", "dma")
    use_pe = variant in ("full", "pe")
    wengs = [getattr(nc, e) for e in W_ENGS]
    xeng = getattr(nc, X_ENG) if not X_ON_ACT else nc.scalar
    oeng = getattr(nc, O_ENG)
    rr = [getattr(nc, e) for e in RR_RINGS]
    rr_cnt = [0]

    def next_ring(default):
        if not rr:
            return default
        e = rr[rr_cnt[0] % len(rr)]
        rr_cnt[0] += 1
        return e
    if variant == "empty":
        nc.vector.memset(ones[:], 1.0)
        return
    odt = BF16 if OUT_BF16 else FP32
    if variant == "dma":
        dummy = spool.tile([COUT, SG * B], odt, tag="dummy")
    wt = xt = st = pt = None
    for pos in range(POS):
        il, j = divmod(pos, WOUT)
        if pos % XG == 0:
            xt = xpool.tile([128, XG * XW], FP8E3 if X_FP8 else BF16)
            if use_dma:
                next_ring(xeng).dma_start(
                    out=xt[:], in_=xk[:, pos * XW : (pos + XG) * XW]
                )
            else:
                nc.vector.memset(xt[0:1, 0:1], 0)
            if not use_pe:
                nc.vector.tensor_copy(out=dummy[0:32, 0:64], in_=xt[0:32, 0:64])
        if pos % WG == 0:
            wt = wpool.tile([128, WG * WW], FP8E3 if W_FP8 else BF16)
            if use_dma:
                weng = next_ring(wengs[(pos // WG) % len(wengs)])
                weng.dma_start(
                    out=wt[:], in_=wk[:, pos * WW : (pos + WG) * WW]
                )
            else:
                nc.vector.memset(wt[0:1, 0:1], 0)
            if not use_pe:
                nc.vector.tensor_copy(out=dummy[0:32, 64:128], in_=wt[0:32, 0:64])
        if not use_pe:
            if pos == POS - 1:
                nc.scalar.dma_start(out=out[:, 0 : SG * B], in_=dummy[:])
            continue
        if pos % SG == 0:
            st = spool.tile([COUT, SG * B], odt)
        if pos % PG == 0:
            pt = ppool.tile([COUT, PG * B], FP32)

        xo = (pos % XG) * XW
        wo = (pos % WG) * WW
        po = (pos % PG) * B

        def emit_mm(w_ap, x_ap, start, stop):
            if SEP_LDW:
                nc.tensor.ldweights(w_ap)
                mm = nc.tensor.matmul(
                    pt[:, po : po + B], w_ap, x_ap, start=start, stop=stop
                )
                mm.ldweights = False
            else:
                nc.tensor.matmul(
                    pt[:, po : po + B], w_ap, x_ap, start=start, stop=stop
                )

        for ck in range(CK):
            emit_mm(
                wt[:, wo + ck * COUT : wo + (ck + 1) * COUT],
                xt[:, xo + ck * B : xo + (ck + 1) * B],
                ck == 0,
                False,
            )
        emit_mm(
            bias_t[0:1, pos * COUT : (pos + 1) * COUT], ones[:], False, True
        )

        if pos % PG == PG - 1:
            so = ((pos - (PG - 1)) % SG) * B
            if X_FP8:
                nc.vector.tensor_scalar_mul(
                    out=st[:, so : so + PG * B], in0=pt[:],
                    scalar1=dsc_t[:, 0:1],
                )
            else:
                nc.vector.tensor_copy(
                    out=st[:, so : so + PG * B], in_=pt[:]
                )
        if pos % SG == SG - 1:
            q0 = (pos - (SG - 1)) * B
            next_ring(oeng).dma_start(
                out=out[:, q0 : q0 + SG * B], in_=st[:]
            )


def _prep_inputs(x, weight, bias):
    """Host-side cast + relayout. Returns per-core input maps."""
    xf = np.asarray(x, dtype=np.float32)
    wf = np.asarray(weight, dtype=np.float32)
    bb = np.asarray(bias, dtype=np.float32)

    # x: [b, c, i, p, j, q] -> [i, j, k=(c,p,q), b]
    x6 = (
        xf.reshape(B, CIN, HOUT, KH, WOUT, KW)
        .transpose(2, 4, 1, 3, 5, 0)
        .reshape(HOUT, WOUT, K, B)
    )
    # w: [i, j, o, c, p, q] -> [i, j, k, o]
    w6 = (
        wf.transpose(0, 1, 3, 4, 5, 2)
        .reshape(HOUT, WOUT, K, COUT)
    )

    dsc_val = 1.0
    if W_FP8 and X_FP8:
        sw = W_TARGET / max(float(np.max(np.abs(wf))), 1e-30)
        ax = np.abs(x6).max(axis=(2, 3))                       # [HOUT, WOUT]
        sx = W_TARGET / np.maximum(ax, 1e-30)
        xb = (x6 * sx[:, :, None, None]).astype(ml_dtypes.float8_e3m4)
        wb = (w6 * (sw / sx)[:, :, None, None]).astype(ml_dtypes.float8_e3m4)
        bb = bb * sw
        dsc_val = 1.0 / sw
    elif W_FP8:
        s = W_TARGET / max(float(np.max(np.abs(wf))), 1e-30)
        xb = (x6 * (1.0 / s)).astype(ml_dtypes.bfloat16)
        wb = (w6 * s).astype(ml_dtypes.float8_e3m4)
    else:
        xb = x6.astype(ml_dtypes.bfloat16)
        wb = w6.astype(ml_dtypes.bfloat16)

    xt = xb.reshape(HOUT, WOUT, CK, 128, B)
    wt = wb.reshape(HOUT, WOUT, CK, 128, COUT)

    dsc = np.full((128, 1), dsc_val, dtype=np.float32)
    in_maps = []
    for c in range(NCORES):
        i0 = c * IPC
        # -> [kp, il, j, ck, {b|o}] so each SBUF partition (kp) reads one
        # long contiguous DRAM run per DMA.
        xc = np.ascontiguousarray(
            xt[i0 : i0 + IPC].transpose(3, 0, 1, 2, 4)
        ).reshape(128, POS * CK * B)
        wc = np.ascontiguousarray(
            wt[i0 : i0 + IPC].transpose(3, 0, 1, 2, 4)
        ).reshape(128, POS * CK * COUT)
        bc = np.ascontiguousarray(bb[i0 : i0 + IPC]).reshape(1, POS * COUT).astype(ml_dtypes.bfloat16)
        m = {"xk": xc, "wk": wc, "bk": bc}
        if X_FP8:
            m["dsc"] = dsc
        in_maps.append(m)
    return in_maps


def _assemble(results):
    out = np.empty((B, COUT, HOUT, WOUT), dtype=np.float32)
    for c in range(NCORES):
        r = np.asarray(results[c]["out"], dtype=np.float32)
        # [o, pos*b] -> [o, il, j, b] -> [b, o, il, j]
        out[:, :, c * IPC : (c + 1) * IPC, :] = (
            r.reshape(COUT, IPC, WOUT, B).transpose(3, 0, 1, 2)
        )
    return out


def _run(inputs, trace=False, **kw):
    in_maps = _prep_inputs(inputs["x"], inputs["weight"], inputs["bias"])
    nc = _build_bass()
    res = run_bass_kernel_spmd(nc, in_maps, list(range(NCORES)), trace=trace, **kw)
    return _assemble(res.results), res


def kernel(**inputs) -> np.ndarray:
    out, _ = _run(inputs, trace=False)
    return out


def _make_exec(nc, in_maps):
    """Build the sharded jitted executable for nc and device-resident args.
    Returns (fn, dev_args)."""
    import jax
    from jax.sharding import Mesh, PartitionSpec
    from jax.experimental.shard_map import shard_map
    from concourse import bass2jax, mybir as mb

    bass2jax.install_neuronx_cc_hook()

    partition_name = (
        nc.partition_id_tensor.name if nc.partition_id_tensor else None
    )
    in_names, out_names, out_avals, zero_outs = [], [], [], []
    for alloc in nc.m.functions[0].allocations:
        if not isinstance(alloc, mb.MemoryLocationSet):
            continue
        name = alloc.memorylocations[0].name
        if alloc.kind == "ExternalInput":
            if name != partition_name:
                in_names.append(name)
        elif alloc.kind == "ExternalOutput":
            out_names.append(name)
            shape = tuple(alloc.tensor_shape)
            dtype = mb.dt.np(alloc.dtype)
            out_avals.append(jax.core.ShapedArray(shape, dtype))
            zero_outs.append(np.zeros(shape, dtype))
    n_params = len(in_names)
    all_in_names = in_names + out_names
    if partition_name is not None:
        all_in_names = all_in_names + [partition_name]

    def _body(*args):
        operands = list(args)
        if partition_name is not None:
            operands.append(bass2jax.partition_id_tensor())
        outs = bass2jax._bass_exec_p.bind(
            *operands,
            out_avals=tuple(out_avals),
            in_names=tuple(all_in_names),
            out_names=tuple(out_names),
            lowering_input_output_aliases=(),
            sim_require_finite=True,
            sim_require_nnan=True,
            nc=nc,
        )
        return tuple(outs)

    devices = jax.devices()[:NCORES]
    mesh = Mesh(np.asarray(devices), ("core",))
    n_outs = len(out_names)
    fn = jax.jit(
        shard_map(
            _body,
            mesh=mesh,
            in_specs=(PartitionSpec("core"),) * (n_params + n_outs),
            out_specs=(PartitionSpec("core"),) * n_outs,
            check_rep=False,
        ),
        keep_unused=True,
    )
    concat_in = [
        np.concatenate([np.asarray(m[name]) for m in in_maps], axis=0)
        for name in in_names
    ]
    concat_zeros = [
        np.zeros((NCORES * z.shape[0], *z.shape[1:]), z.dtype) for z in zero_outs
    ]
    sharding = jax.sharding.NamedSharding(mesh, PartitionSpec("core"))
    dev_in = [jax.device_put(a, sharding) for a in concat_in]
    dev_zeros = [jax.device_put(a, sharding) for a in concat_zeros]
    return fn, dev_in + dev_zeros


def _timed_exec(nc, in_maps, n_iters):
    """Compile nc via the bass2jax path, keep inputs device-resident, and
    return the min wall-clock seconds over n_iters calls."""
    import time

    import jax

    fn, dev_args = _make_exec(nc, in_maps)
    # warmup (compiles)
    r = fn(*dev_args)
    jax.block_until_ready(r)
    times = []
    for _ in range(n_iters):
        t0 = time.perf_counter()
        r = fn(*dev_args)
        jax.block_until_ready(r)
        times.append(time.perf_counter() - t0)
    print(f"    raw times (ms): {[f'{t * 1e3:.2f}' for t in times]}")
    # median: the axon dispatch constant is bimodal (~60ms rare / ~100ms
    # typical), so min() is a trap; medians are tight (+-0.5ms).
    return float(np.median(times)), r


def bench(inputs, r_small=1, r_big=1001, n_iters=21, variant="full"):
    """Estimate per-kernel HW time by differencing two repeat counts.
    r_big=1001 makes the signal (1000 x T_kernel) ~10x the few-ms axon
    dispatch jitter, so the derived per-kernel time is good to ~1 us."""
    in_maps = _prep_inputs(inputs["x"], inputs["weight"], inputs["bias"])
    t_small, _ = _timed_exec(_build_bass(repeat=r_small, variant=variant), in_maps, n_iters)
    t_big, _ = _timed_exec(_build_bass(repeat=r_big, variant=variant), in_maps, n_iters)
    ns = (t_big - t_small) / (r_big - r_small) * 1e9
    print(
        f"bench: T({r_small})={t_small * 1e3:.3f} ms  T({r_big})={t_big * 1e3:.3f} ms"
        f"  -> per-kernel {ns:.0f} ns"
    )
    return ns



# revision 51
# speedup vs baseline: 1.0303x; 1.0303x over previous
"""LocallyConnected2d (non-overlapping 3x3 patches) Trainium2 kernel.

Problem: x [B=32, Cin=128, H=96, W=96], weight [Hout=32, Wout=32, Cout=128,
Cin=128, 3, 3], bias [Hout, Wout, Cout] -> out [B, Cout, Hout, Wout].

For each of the 1024 output positions (i, j) this is an independent
[B=32, K=1152] x [K=1152, Cout=128] matmul (K = Cin*KH*KW) plus bias.

Strategy (measured facts in parentheses; all timing via repeat-loop
differencing, (T(1001)-T(1))/1000, which is good to ~1 us):
  - Shard the 1024 positions over 8 NeuronCores by Hout rows (4 rows =
    128 positions per core).  Every weight byte is used exactly once, so
    position-sharding splits the dominant tensor evenly with zero
    duplication and needs no collectives.
  - The binding constraint is per-core DMA-in bandwidth, ~356 GB/s TOTAL
    regardless of how many HWDGE rings carry it (measured: splitting
    w-DMAs across sync+scalar+gpsimd made it slower, not faster).  So the
    whole game is shrinking input bytes:
      * weight -> fp8e3 (e3m4), one global scale sw = 14/absmax(w).
      * x -> fp8e3 with per-position scales sx[pos] = 14/absmax(patch),
        folded into the WEIGHT quantization (w8 = e3m4(w*sw/sx[pos])) so
        PSUM = sw*(x.w) for every position and a single global descale
        1/sw suffices.  (Exact host sim of the harness data: rel_max
        1.65e-2 vs the 2e-2 gate; hardware reproduces the sim value.)
    Per-core input drops 47.2 MB (bf16) -> 23.6 MB -> ~66-72 us DMA.
  - Layouts put the contraction k = c*9+p*3+q on SBUF partitions
    (k = ck*128 + kp):
        wk [kp=128, pos, ck=9, o=128]  fp8e3
        xk [kp=128, pos, ck=9, b=32]   fp8e3
    so every DMA descriptor is a long contiguous run per partition.
  - Per position: 9 matmuls (lhsT = w chunk [128k x 128o] stationary,
    rhs = x chunk [128k x 32b] moving) accumulate into a PSUM bank slice.
    One accumulation group per position: start/stop flags cost ~76 PE
    cycles per event (measured), so they are amortized over the group.
    The old 10th "bias matmul" ([1,128] stationary x ones) measured a
    shocking ~145 ns each and is gone:
  - Bias ships as [Cout=128 partitions, pos] fp32 (the previous
    [1, pos*Cout] layout DMA'd at 1/128th ring speed = 12.6 us) and is
    applied by the DVE: one fused scalar_tensor_tensor per PSUM bank,
        st = (psum * (1/sw)) + bias_bcast    (bias broadcast along b
    with a stride-0 AP).  16 positions share one PSUM bank [128, 512].
  - Rings: w tiles on sync; x tiles + bias + output stores on scalar
    (stores on the gpsimd SW-DGE ring measurably dragged the pipeline);
    deep tile pools (8 w bufs, 4 x bufs) keep the DMA queues from ever
    waiting on compute.
  - Mixed-dtype matmul (fp8e3 stationary x bf16 moving) works on TRN2;
    fp8e3 x fp8e3 is what ships.  Separate InstLdweights + non-self-
    loading InstMatmult measured 2.4x SLOWER (per-instruction decode
    overhead) — keep self-loading matmuls.
"""

import numpy as np
import ml_dtypes

import concourse.bass as bass
import concourse.bacc as bacc
import concourse.mybir as mybir
import concourse.tile as tile
from concourse.bass_utils import run_bass_kernel_spmd

KH = KW = 3
B, CIN, H, W_IN = 32, 128, 96, 96
HOUT, WOUT, COUT = 32, 32, 128
NCORES = 8
IPC = HOUT // NCORES          # Hout rows per core = 4
POS = IPC * WOUT              # positions per core = 128
K = CIN * KH * KW             # 1152
CK = K // 128                 # 9 k-chunks of 128

WG = 8     # positions per weight-DMA tile
XG = 16    # positions per x-DMA tile
PG = 16    # positions per PSUM bank
SG = 32    # positions per output staging tile
WBUFS = 8  # weight pool buffers (deep prefetch decouples DMA from PE stalls)
XBUFS = 4  # x pool buffers
PBUFS = 6  # PSUM pool buffers
X_ON_ACT = False  # legacy: issue x DMAs on the scalar (ACT) HWDGE ring
# Ring layout: per-core input DMA bandwidth is capped at ~356 GB/s TOTAL
# (shared across queues — measured: splitting rings does not scale), so the
# split below is about ordering, not bandwidth: w tiles never queue behind
# x tiles or behind output stores that wait on compute.
W_ENGS = ("sync",)   # weight DMAs: sync HWDGE ring only
X_ENG = "scalar"     # x DMAs: scalar HWDGE ring
O_ENG = "scalar"     # output stores: scalar ring has ~40us of slack vs sync
# Unified round-robin: every input DMA tile (w and x alike) takes the next
# ring in RR_RINGS, balancing bytes across rings.  Overrides W_ENGS/X_ENG
# when non-empty.  Each HWDGE ring sustains ~356 GB/s; the weight stream
# alone needs ~3 rings to keep up with the PE.
RR_RINGS = ()

BF16 = mybir.dt.bfloat16
FP32 = mybir.dt.float32
FP8E3 = mybir.dt.float8e3  # e3m4, max 15.5

# Weight dtype: fp8e3 halves the dominant weight DMA stream.  A single
# global scale s = W_TARGET/absmax(w) is applied to w before the fp8 cast
# and its inverse is folded into the bf16 x on the host, so the PE
# accumulates the true x.w in PSUM and nothing on-chip changes.
# Measured accuracy (exact host sim): rel_max 1.16e-2 vs 2e-2 gate.
W_FP8 = True
W_TARGET = 14.0
OUT_BF16 = False  # stage + store the output in bf16 (halves store bytes)
SEP_LDW = False   # separate InstLdweights + non-self-loading InstMatmult
# x in fp8e3 as well: per-position scale sx[pos]=14/absmax(x_patch) folded
# into w's quantization scale (w8 = e3m4(w*sw/sx[pos])), so PSUM = sw*(x.w)
# and a single global descale 1/sw (shipped as a [128,1] input, applied by
# the DVE tensor_scalar_mul on the PSUM->SBUF copy) recovers the output.
# Bias rides the PE at scale sw (bias_q = bias*sw).  Exact host sim:
# rel_max 1.65e-2 vs the 2e-2 gate.
X_FP8 = True
# Bias via DVE instead of a per-position PE matmul: bias ships as a
# [COUT, POS] fp32 tile (fast parallel DMA; the old [1, POS*COUT] layout
# crawls at 1/128th ring bandwidth) and the PSUM->SBUF copy becomes one
# fused DVE scalar_tensor_tensor: st = (psum * dsc) + bias_bcast.
BIAS_DVE = True
# Filler matmuls per position: dummy self-contained matmuls into a scratch
# PSUM tile that is never read.  They keep the PE busy through the sub-us
# DMA-wait gaps so its DVFS ramp reaches full clock (idle resets the ramp;
# at the mid p-state the PE rides just above the DMA rate and becomes the
# critical path).  Extra PE work must stay under the DMA period.
FILLER = 0

_NC_CACHE = {}


def set_config(**kw):
    g = globals()
    for k, v in kw.items():
        assert k in g, k
        g[k] = v
    _NC_CACHE.clear()


def _config_key():
    return (WG, XG, PG, SG, WBUFS, XBUFS, PBUFS, X_ON_ACT, W_FP8, W_ENGS,
            X_ENG, O_ENG, RR_RINGS, OUT_BF16, SEP_LDW, X_FP8, BIAS_DVE,
            FILLER)


def _build_bass(repeat=1, variant="full"):
    """Build the Bass program. repeat>1 wraps the body in a dynamic loop
    (identical work each trip) so wall-clock timing can amortize the axon
    dispatch overhead: T(repeat) ~= overhead + repeat * T_kernel.
    variant: "full" | "dma" (input DMAs only) | "pe" (no input DMAs) |
    "empty" (loop overhead calibration)."""
    key = ("nc", repeat, variant, _config_key())
    if key in _NC_CACHE:
        return _NC_CACHE[key]
    nc = bacc.Bacc()
    wdt = FP8E3 if W_FP8 else BF16
    xdt = FP8E3 if X_FP8 else BF16
    xk = nc.declare_dram_parameter("xk", [128, POS * CK * B], xdt, isOutput=False)
    wk = nc.declare_dram_parameter("wk", [128, POS * CK * COUT], wdt, isOutput=False)
    if BIAS_DVE:
        bk = nc.declare_dram_parameter("bk", [COUT, POS], FP32, isOutput=False)
    else:
        bk = nc.declare_dram_parameter("bk", [1, POS * COUT], BF16, isOutput=False)
    dsc = (
        nc.declare_dram_parameter("dsc", [128, 1], FP32, isOutput=False)
        if X_FP8
        else None
    )
    out = nc.declare_dram_parameter(
        "out", [COUT, POS * B], BF16 if OUT_BF16 else FP32, isOutput=True
    )

    XW = CK * B      # x columns per position = 288
    WW = CK * COUT   # w columns per position = 1152

    with tile.TileContext(nc) as tc:
        with (
            tc.tile_pool(name="wpool", bufs=WBUFS) as wpool,
            tc.tile_pool(name="xpool", bufs=XBUFS) as xpool,
            tc.tile_pool(name="spool", bufs=2) as spool,
            tc.tile_pool(name="cpool", bufs=1) as cpool,
            tc.tile_pool(name="ppool", bufs=PBUFS, space="PSUM") as ppool,
            tc.tile_pool(name="fpool", bufs=1, space="PSUM") as fpool,
        ):
            fpt = (
                fpool.tile([COUT, B], FP32, name="fpt") if FILLER else None
            )
            if BIAS_DVE:
                ones = None
                bias_t = cpool.tile([COUT, POS], FP32)
                nc.scalar.dma_start(out=bias_t[:], in_=bk[:])
            else:
                ones = cpool.tile([1, B], BF16)
                nc.vector.memset(ones[:], 1.0)
                bias_t = cpool.tile([1, POS * COUT], BF16)
                # [1, N] DMAs run at 1/128th of ring bandwidth (single
                # partition line): keep this off the w/x rings.
                nc.gpsimd.dma_start(out=bias_t[:], in_=bk[:])
            dsc_t = None
            if dsc is not None:
                dsc_t = cpool.tile([128, 1], FP32)
                nc.gpsimd.dma_start(out=dsc_t[:], in_=dsc[:])

            def body():
                _emit_body(nc, tc, xk, wk, out, wpool, xpool, spool, ppool,
                           ones, bias_t, dsc_t, fpt, variant)

            if repeat == 1:
                body()
            else:
                with tc.For_i(0, repeat, 1):
                    body()
    nc.finalize()
    _NC_CACHE[key] = nc
    return nc


def _emit_body(nc, tc, xk, wk, out, wpool, xpool, spool, ppool, ones, bias_t,
               dsc_t, fpt, variant="full"):
    XW = CK * B
    WW = CK * COUT
    use_dma = variant in ("full", "dma")
    use_pe = variant in ("full", "pe")
    wengs = [getattr(nc, e) for e in W_ENGS]
    xeng = getattr(nc, X_ENG) if not X_ON_ACT else nc.scalar
    oeng = getattr(nc, O_ENG)
    rr = [getattr(nc, e) for e in RR_RINGS]
    rr_cnt = [0]

    def next_ring(default):
        if not rr:
            return default
        e = rr[rr_cnt[0] % len(rr)]
        rr_cnt[0] += 1
        return e
    if variant == "empty":
        nc.vector.memset(ones[:], 1.0)
        return
    odt = BF16 if OUT_BF16 else FP32
    if variant == "dma":
        dummy = spool.tile([COUT, SG * B], odt, tag="dummy")
    wt = xt = st = pt = None
    for pos in range(POS):
        il, j = divmod(pos, WOUT)
        if pos % XG == 0:
            xt = xpool.tile([128, XG * XW], FP8E3 if X_FP8 else BF16)
            if use_dma:
                next_ring(xeng).dma_start(
                    out=xt[:], in_=xk[:, pos * XW : (pos + XG) * XW]
                )
            else:
                nc.vector.memset(xt[0:1, 0:1], 0)
            if not use_pe:
                nc.vector.tensor_copy(out=dummy[0:32, 0:64], in_=xt[0:32, 0:64])
        if pos % WG == 0:
            wt = wpool.tile([128, WG * WW], FP8E3 if W_FP8 else BF16)
            if use_dma:
                weng = next_ring(wengs[(pos // WG) % len(wengs)])
                weng.dma_start(
                    out=wt[:], in_=wk[:, pos * WW : (pos + WG) * WW]
                )
            else:
                nc.vector.memset(wt[0:1, 0:1], 0)
            if not use_pe:
                nc.vector.tensor_copy(out=dummy[0:32, 64:128], in_=wt[0:32, 0:64])
        if not use_pe:
            if pos == POS - 1:
                nc.scalar.dma_start(out=out[:, 0 : SG * B], in_=dummy[:])
            continue
        if pos % SG == 0:
            st = spool.tile([COUT, SG * B], odt)
        if pos % PG == 0:
            pt = ppool.tile([COUT, PG * B], FP32)

        xo = (pos % XG) * XW
        wo = (pos % WG) * WW
        po = (pos % PG) * B

        def emit_mm(w_ap, x_ap, start, stop):
            if SEP_LDW:
                nc.tensor.ldweights(w_ap)
                mm = nc.tensor.matmul(
                    pt[:, po : po + B], w_ap, x_ap, start=start, stop=stop
                )
                mm.ldweights = False
            else:
                nc.tensor.matmul(
                    pt[:, po : po + B], w_ap, x_ap, start=start, stop=stop
                )

        for ck in range(CK):
            emit_mm(
                wt[:, wo + ck * COUT : wo + (ck + 1) * COUT],
                xt[:, xo + ck * B : xo + (ck + 1) * B],
                ck == 0,
                BIAS_DVE and ck == CK - 1,
            )
        if not BIAS_DVE:
            emit_mm(
                bias_t[0:1, pos * COUT : (pos + 1) * COUT], ones[:], False, True
            )
        for _ in range(FILLER):
            nc.tensor.matmul(
                fpt[:], wt[:, wo : wo + COUT], xt[:, xo : xo + B],
                start=True, stop=True, skip_group_check=True,
            )

        if pos % PG == PG - 1:
            p0 = pos - (PG - 1)
            so = (p0 % SG) * B
            st2 = st[:, so : so + PG * B]
            if BIAS_DVE:
                # st = (psum * dsc) + bias, bias broadcast along b (stride-0)
                b2 = bias_t[:, p0 : p0 + PG]
                b3 = bass.AP(b2.tensor, b2.offset, list(b2.ap) + [[0, B]])
                st3 = bass.AP(st2.tensor, st2.offset, [st2.ap[0], [B, PG], [1, B]])
                pt3 = bass.AP(pt[:].tensor, pt[:].offset,
                              [pt[:].ap[0], [B, PG], [1, B]])
                nc.vector.scalar_tensor_tensor(
                    out=st3, in0=pt3,
                    scalar=dsc_t[:, 0:1] if X_FP8 else 1.0,
                    in1=b3, op0=mybir.AluOpType.mult, op1=mybir.AluOpType.add,
                )
            elif X_FP8:
                nc.vector.tensor_scalar_mul(
                    out=st2, in0=pt[:], scalar1=dsc_t[:, 0:1],
                )
            else:
                nc.vector.tensor_copy(out=st2, in_=pt[:])
        if pos % SG == SG - 1:
            q0 = (pos - (SG - 1)) * B
            next_ring(oeng).dma_start(
                out=out[:, q0 : q0 + SG * B], in_=st[:]
            )


def _prep_inputs(x, weight, bias):
    """Host-side cast + relayout. Returns per-core input maps."""
    xf = np.asarray(x, dtype=np.float32)
    wf = np.asarray(weight, dtype=np.float32)
    bb = np.asarray(bias, dtype=np.float32)

    # x: [b, c, i, p, j, q] -> [i, j, k=(c,p,q), b]
    x6 = (
        xf.reshape(B, CIN, HOUT, KH, WOUT, KW)
        .transpose(2, 4, 1, 3, 5, 0)
        .reshape(HOUT, WOUT, K, B)
    )
    # w: [i, j, o, c, p, q] -> [i, j, k, o]
    w6 = (
        wf.transpose(0, 1, 3, 4, 5, 2)
        .reshape(HOUT, WOUT, K, COUT)
    )

    dsc_val = 1.0
    if W_FP8 and X_FP8:
        sw = W_TARGET / max(float(np.max(np.abs(wf))), 1e-30)
        ax = np.abs(x6).max(axis=(2, 3))                       # [HOUT, WOUT]
        sx = W_TARGET / np.maximum(ax, 1e-30)
        xb = (x6 * sx[:, :, None, None]).astype(ml_dtypes.float8_e3m4)
        wb = (w6 * (sw / sx)[:, :, None, None]).astype(ml_dtypes.float8_e3m4)
        if not BIAS_DVE:
            bb = bb * sw  # bias rides the PE at PSUM scale sw
        dsc_val = 1.0 / sw
    elif W_FP8:
        s = W_TARGET / max(float(np.max(np.abs(wf))), 1e-30)
        xb = (x6 * (1.0 / s)).astype(ml_dtypes.bfloat16)
        wb = (w6 * s).astype(ml_dtypes.float8_e3m4)
    else:
        xb = x6.astype(ml_dtypes.bfloat16)
        wb = w6.astype(ml_dtypes.bfloat16)

    xt = xb.reshape(HOUT, WOUT, CK, 128, B)
    wt = wb.reshape(HOUT, WOUT, CK, 128, COUT)

    dsc = np.full((128, 1), dsc_val, dtype=np.float32)
    in_maps = []
    for c in range(NCORES):
        i0 = c * IPC
        # -> [kp, il, j, ck, {b|o}] so each SBUF partition (kp) reads one
        # long contiguous DRAM run per DMA.
        xc = np.ascontiguousarray(
            xt[i0 : i0 + IPC].transpose(3, 0, 1, 2, 4)
        ).reshape(128, POS * CK * B)
        wc = np.ascontiguousarray(
            wt[i0 : i0 + IPC].transpose(3, 0, 1, 2, 4)
        ).reshape(128, POS * CK * COUT)
        if BIAS_DVE:
            bc = np.ascontiguousarray(
                bb[i0 : i0 + IPC].reshape(POS, COUT).T
            ).astype(np.float32)
        else:
            bc = (
                np.ascontiguousarray(bb[i0 : i0 + IPC])
                .reshape(1, POS * COUT)
                .astype(ml_dtypes.bfloat16)
            )
        m = {"xk": xc, "wk": wc, "bk": bc}
        if X_FP8:
            m["dsc"] = dsc
        in_maps.append(m)
    return in_maps


def _assemble(results):
    out = np.empty((B, COUT, HOUT, WOUT), dtype=np.float32)
    for c in range(NCORES):
        r = np.asarray(results[c]["out"], dtype=np.float32)
        # [o, pos*b] -> [o, il, j, b] -> [b, o, il, j]
        out[:, :, c * IPC : (c + 1) * IPC, :] = (
            r.reshape(COUT, IPC, WOUT, B).transpose(3, 0, 1, 2)
        )
    return out


def _run(inputs, trace=False, **kw):
    in_maps = _prep_inputs(inputs["x"], inputs["weight"], inputs["bias"])
    nc = _build_bass()
    res = run_bass_kernel_spmd(nc, in_maps, list(range(NCORES)), trace=trace, **kw)
    return _assemble(res.results), res


def kernel(**inputs) -> np.ndarray:
    out, _ = _run(inputs, trace=False)
    return out


def _make_exec(nc, in_maps):
    """Build the sharded jitted executable for nc and device-resident args.
    Returns (fn, dev_args)."""
    import jax
    from jax.sharding import Mesh, PartitionSpec
    from jax.experimental.shard_map import shard_map
    from concourse import bass2jax, mybir as mb

    bass2jax.install_neuronx_cc_hook()

    partition_name = (
        nc.partition_id_tensor.name if nc.partition_id_tensor else None
    )
    in_names, out_names, out_avals, zero_outs = [], [], [], []
    for alloc in nc.m.functions[0].allocations:
        if not isinstance(alloc, mb.MemoryLocationSet):
            continue
        name = alloc.memorylocations[0].name
        if alloc.kind == "ExternalInput":
            if name != partition_name:
                in_names.append(name)
        elif alloc.kind == "ExternalOutput":
            out_names.append(name)
            shape = tuple(alloc.tensor_shape)
            dtype = mb.dt.np(alloc.dtype)
            out_avals.append(jax.core.ShapedArray(shape, dtype))
            zero_outs.append(np.zeros(shape, dtype))
    n_params = len(in_names)
    all_in_names = in_names + out_names
    if partition_name is not None:
        all_in_names = all_in_names + [partition_name]

    def _body(*args):
        operands = list(args)
        if partition_name is not None:
            operands.append(bass2jax.partition_id_tensor())
        outs = bass2jax._bass_exec_p.bind(
            *operands,
            out_avals=tuple(out_avals),
            in_names=tuple(all_in_names),
            out_names=tuple(out_names),
            lowering_input_output_aliases=(),
            sim_require_finite=True,
            sim_require_nnan=True,
            nc=nc,
        )
        return tuple(outs)

    devices = jax.devices()[:NCORES]
    mesh = Mesh(np.asarray(devices), ("core",))
    n_outs = len(out_names)
    fn = jax.jit(
        shard_map(
            _body,
            mesh=mesh,
            in_specs=(PartitionSpec("core"),) * (n_params + n_outs),
            out_specs=(PartitionSpec("core"),) * n_outs,
            check_rep=False,
        ),
        keep_unused=True,
    )
    concat_in = [
        np.concatenate([np.asarray(m[name]) for m in in_maps], axis=0)
        for name in in_names
    ]
    concat_zeros = [
        np.zeros((NCORES * z.shape[0], *z.shape[1:]), z.dtype) for z in zero_outs
    ]
    sharding = jax.sharding.NamedSharding(mesh, PartitionSpec("core"))
    dev_in = [jax.device_put(a, sharding) for a in concat_in]
    dev_zeros = [jax.device_put(a, sharding) for a in concat_zeros]
    return fn, dev_in + dev_zeros


def _timed_exec(nc, in_maps, n_iters):
    """Compile nc via the bass2jax path, keep inputs device-resident, and
    return the min wall-clock seconds over n_iters calls."""
    import time

    import jax

    fn, dev_args = _make_exec(nc, in_maps)
    # warmup (compiles)
    r = fn(*dev_args)
    jax.block_until_ready(r)
    times = []
    for _ in range(n_iters):
        t0 = time.perf_counter()
        r = fn(*dev_args)
        jax.block_until_ready(r)
        times.append(time.perf_counter() - t0)
    print(f"    raw times (ms): {[f'{t * 1e3:.2f}' for t in times]}")
    # median: the axon dispatch constant is bimodal (~60ms rare / ~100ms
    # typical), so min() is a trap; medians are tight (+-0.5ms).
    return float(np.median(times)), r


def bench(inputs, r_small=1, r_big=1001, n_iters=21, variant="full"):
    """Estimate per-kernel HW time by differencing two repeat counts.
    r_big=1001 makes the signal (1000 x T_kernel) ~10x the few-ms axon
    dispatch jitter, so the derived per-kernel time is good to ~1 us."""
    in_maps = _prep_inputs(inputs["x"], inputs["weight"], inputs["bias"])
    t_small, _ = _timed_exec(_build_bass(repeat=r_small, variant=variant), in_maps, n_iters)
    t_big, _ = _timed_exec(_build_bass(repeat=r_big, variant=variant), in_maps, n_iters)
    ns = (t_big - t_small) / (r_big - r_small) * 1e9
    print(
        f"bench: T({r_small})={t_small * 1e3:.3f} ms  T({r_big})={t_big * 1e3:.3f} ms"
        f"  -> per-kernel {ns:.0f} ns"
    )
    return ns

